# revision 10
# baseline (speedup 1.0000x reference)
"""Multi-head attention (B=2, C=256, H=W=64, nh=8) on 8 trn2 NeuronCores.

Sharding: batch*heads = 16 (b,h) pairs -> 2 pairs per core.
Core c handles batch b = c//4 and heads {2*(c%4), 2*(c%4)+1}.
Each core computes q/k/v projections for its heads, transposed-layout
attention (scores kept as [t, s] so softmax needs no transposes), and its
partial output projection; the host sums the 4 per-batch partials + bias.

Layout notes (per core, S = 4096, hd = 32):
  x_b  [128, 8192]  x[b] (bf16) as two 128-channel chunks side by side
  qT4  [128, 4096]  q^T for head h replicated on 4 partition strips
                    (row-packed K=32 scores matmuls need weights and rhs at
                     the same base partition, 32*i for strip i); bf16
  kT4  [128, 4096]  k^T strips: strip i holds t-tiles with tt % 4 == i
                    (kT4[32i:32i+32, 128g:128g+128] = kT[:, 128*(4g+i):...])
  v_sb [128, 2112]  32 t-tiles x 66 cols: [v_h0 | ones | v_h1 | ones] bf16
                    (ones column computes the softmax denominator for free)
  scores psum [128, 2048]: 4 banks, t-tiles 4g..4g+3 of one s-chunk; one
                    Exp ACTIVATE over all 2048 columns -> pT (bf16)
  attn@v: M=33 (v|ones) matmuls accumulated into two psum banks (even/odd
                    t-tiles) so each bank's first write can use start=True
  out  [256, 4096]  partial conv1x1 output for batch b (2 heads' worth),
                    DMAed straight from PSUM
"""

import sys

for _p in ("/opt/trn_rl_repo",):
    if _p not in sys.path:
        sys.path.append(_p)

import numpy as np

B, C, HH, WW = 2, 256, 64, 64
S = HH * WW            # 4096
NH = 8
HD = C // NH           # 32
P = 128
NJ = S // 512          # 8 s-chunks
NT = S // P            # 32 t-tiles
NG = NT // 4           # 8 t-groups
SCALE = 1.0 / np.sqrt(np.float32(HD))

_BUILT = None


def _build():
    import concourse.bass as bass
    import concourse.mybir as mybir
    import concourse.tile as tile
    from concourse import bacc

    dt = mybir.dt
    f32 = dt.float32
    f32r = dt.float32r
    bf16 = dt.bfloat16
    AF = mybir.ActivationFunctionType

    nc = bacc.Bacc("TRN2", target_bir_lowering=False, debug=False, num_devices=8)

    x2 = nc.dram_tensor("x2", [P, 2 * S], f32, kind="ExternalInput")
    wq_d = nc.dram_tensor("wq_sb", [P, 128], f32, kind="ExternalInput")
    wk_d = nc.dram_tensor("wk_sb", [P, 128], f32, kind="ExternalInput")
    wv_d = nc.dram_tensor("wv_sb", [P, 128], f32, kind="ExternalInput")
    wo_d = nc.dram_tensor("wo_sb", [64, 256], f32, kind="ExternalInput")
    bqkf_d = nc.dram_tensor("bqkf", [P, 512], f32, kind="ExternalInput")
    bvf_d = nc.dram_tensor("bvf", [P, 64], f32, kind="ExternalInput")
    out_d = nc.dram_tensor("out", [C, S], f32, kind="ExternalOutput")

    with tile.TileContext(nc) as tc:
        with (
            tc.tile_pool(name="const", bufs=1) as cpool,
            tc.tile_pool(name="qk", bufs=1) as qkpool,
            tc.tile_pool(name="pt", bufs=3) as ptpool,
            tc.tile_pool(name="work", bufs=4) as wpool,
            tc.tile_pool(name="stage", bufs=4) as spool,
            tc.tile_pool(name="ps_sc", bufs=1, space="PSUM") as ps_sc,
            tc.tile_pool(name="ps_acc", bufs=1, space="PSUM") as ps_acc,
            tc.tile_pool(name="ps_prj", bufs=2, space="PSUM") as ps_prj,
        ):
            # ---- constants / inputs in SBUF ----
            x_sb = cpool.tile([P, 2 * S], f32)
            x_b = cpool.tile([P, 2 * S], bf16)
            for ck in range(2):
                for piece in range(2):
                    sl = slice(ck * S + piece * 2048, ck * S + (piece + 1) * 2048)
                    nc.sync.dma_start(x_sb[:, sl], x2[:, sl])
                    nc.vector.tensor_copy(x_b[:, sl], x_sb[:, sl])
            wq_t = cpool.tile([P, 128], f32)
            wk_t = cpool.tile([P, 128], f32)
            wv_t = cpool.tile([P, 128], f32)
            nc.sync.dma_start(wq_t[:], wq_d[:])
            nc.sync.dma_start(wk_t[:], wk_d[:])
            nc.sync.dma_start(wv_t[:], wv_d[:])
            wq_b = cpool.tile([P, 128], bf16)
            wk_b = cpool.tile([P, 128], bf16)
            wv_b = cpool.tile([P, 128], bf16)
            nc.vector.tensor_copy(wq_b[:], wq_t[:])
            nc.vector.tensor_copy(wk_b[:], wk_t[:])
            nc.vector.tensor_copy(wv_b[:], wv_t[:])
            wo_t = cpool.tile([64, 256], f32)
            nc.sync.dma_start(wo_t[:], wo_d[:])
            wo_r = cpool.tile([64, 256], f32r)
            nc.vector.tensor_copy(wo_r[:], wo_t[:])
            bqkf_t = cpool.tile([P, 512], f32)
            bvf_t = cpool.tile([P, 64], f32)
            nc.sync.dma_start(bqkf_t[:], bqkf_d[:])
            nc.sync.dma_start(bvf_t[:], bvf_d[:])

            # ---- v projection: v_sb [128, 32*66] (t on partitions), bf16 ----
            v_sb = cpool.tile([P, NT * 66], bf16)
            ones32 = cpool.tile([P, NT, 1], f32)
            nc.vector.memset(ones32[:], 1.0)
            v3 = v_sb[:].rearrange("p (t c) -> p t c", c=66)
            nc.vector.tensor_copy(v3[:, :, 32:33], ones32[:])
            nc.vector.tensor_copy(v3[:, :, 65:66], ones32[:])
            for tt in range(NT):
                pv = ps_prj.tile([P, 64], f32, tag="prj")
                for ck in range(2):
                    nc.tensor.matmul(
                        pv[:],
                        x_b[:, ck * S + tt * P : ck * S + (tt + 1) * P],
                        wv_b[:, 64 * ck : 64 * ck + 64],
                        start=(ck == 0),
                        stop=(ck == 1),
                    )
                base = tt * 66
                nc.vector.tensor_add(v_sb[:, base : base + 32], pv[:, 0:32], bvf_t[:, 0:32])
                nc.vector.tensor_add(v_sb[:, base + 33 : base + 65], pv[:, 32:64], bvf_t[:, 32:64])

            # ---- q/k projections for both heads: qT4/kT4 [128, 4096] bf16 ----
            # One DVE bias-add into strip 0 (q) / a staging tile (k), then DMA
            # replication/distribution to the other strips (keeps DVE load low).
            qT4 = [qkpool.tile([P, S], bf16, tag=f"qT4_{h}", name=f"qT4_{h}") for h in range(2)]
            kT4 = [qkpool.tile([P, S], bf16, tag=f"kT4_{h}", name=f"kT4_{h}") for h in range(2)]
            # per j: one col-packed wave computes q/k for BOTH heads into one
            # psum bank (rows: q_h0 | k_h0 | q_h1 | k_h1), one combined
            # bias-add, then DMA distribution into the qT4/kT4 strips
            for j in range(NJ):
                sj = slice(512 * j, 512 * (j + 1))
                pqk = ps_prj.tile([P, 512], f32, tag="prj", name=f"pqk_{j}", bufs=2)
                for ck in range(2):
                    for s4, wsel in enumerate((wq_b, wk_b, wq_b, wk_b)):
                        hcol = 32 * (s4 // 2)
                        nc.tensor.matmul(
                            pqk[32 * s4 : 32 * s4 + 32, :],
                            wsel[:, 64 * ck + hcol : 64 * ck + hcol + 32],
                            x_b[:, ck * S + 512 * j : ck * S + 512 * (j + 1)],
                            tile_position=(0, 32 * s4),
                            start=(ck == 0),
                            stop=(ck == 1),
                        )
                qkst = spool.tile([P, 512], bf16, tag="qkst")
                nc.vector.tensor_add(qkst[:], pqk[:], bqkf_t[:])
                for h in range(2):
                    for i in range(4):
                        nc.sync.dma_start(qT4[h][32 * i : 32 * i + 32, sj], qkst[64 * h : 64 * h + 32, :])
                        nc.sync.dma_start(
                            kT4[h][32 * i : 32 * i + 32, 128 * j : 128 * (j + 1)],
                            qkst[64 * h + 32 : 64 * h + 64, 128 * i : 128 * (i + 1)],
                        )

            # ---- attention + output projection ----
            attnT = cpool.tile([64, S], f32r)
            for j in range(NJ):
                sj = slice(512 * j, 512 * (j + 1))
                for h in range(2):
                    accA = ps_acc.tile([P, 512], f32, tag="accA")
                    accB = ps_acc.tile([P, 512], f32, tag="accB")

                    # half-groups of 2 t-tiles: half (g, u) covers t-tiles
                    # 4g+2u, 4g+2u+1 (strips 2u, 2u+1), psum tags scA/scB
                    # ping-pong so exp of one half overlaps scores of the next
                    def scores_half(g, u):
                        sc = ps_sc.tile(
                            [P, 1024], f32, tag=f"sc{u}", name=f"sc_{h}_{g}_{u}"
                        )
                        for ii in range(2):
                            i = 2 * u + ii
                            nc.tensor.matmul(
                                sc[:, 512 * ii : 512 * (ii + 1)],
                                kT4[h][32 * i : 32 * i + 32, 128 * g : 128 * (g + 1)],
                                qT4[h][32 * i : 32 * i + 32, sj],
                                tile_position=(32 * i, 0),
                            )
                        return sc

                    def attnv_half(g, u, pt):
                        for ii in range(2):
                            tt = 4 * g + 2 * u + ii
                            acc, cpos = (accA, 0) if tt % 2 == 0 else (accB, 64)
                            nc.tensor.matmul(
                                acc[cpos : cpos + 33, :],
                                v_sb[:, tt * 66 + 33 * h : tt * 66 + 33 * h + 33],
                                pt[:, 512 * ii : 512 * (ii + 1)],
                                tile_position=(0, cpos),
                                start=(tt < 2),
                                stop=(tt >= NT - 2),
                            )

                    halves = [(g, u) for g in range(NG) for u in range(2)]
                    sc = scores_half(*halves[0])
                    for idx, (g, u) in enumerate(halves):
                        pt = ptpool.tile([P, 1024], bf16, tag="pt")
                        nc.scalar.activation(pt[:], sc[:], AF.Exp, scale=float(SCALE))
                        if idx + 1 < len(halves):
                            sc = scores_half(*halves[idx + 1])
                        attnv_half(g, u, pt)
                    # combine strips, normalize, write attnT rows for head h
                    numT = wpool.tile([33, 512], f32, tag="numT")
                    nc.vector.tensor_copy(numT[:], accB[64:97, :])
                    nc.vector.tensor_add(numT[:], numT[:], accA[0:33, :])
                    rec = wpool.tile([1, 512], f32, tag="rec")
                    nc.vector.reciprocal(rec[:], numT[32:33, :])
                    bc = wpool.tile([32, 512], f32, tag="bc")
                    nc.gpsimd.partition_broadcast(bc[:], rec[:])
                    nc.vector.tensor_mul(attnT[32 * h : 32 * h + 32, sj], numT[0:32, :], bc[:])

            # ---- output projection tail (attnT fully built) ----
            for j in range(NJ):
                sj = slice(512 * j, 512 * (j + 1))
                for m in range(2):
                    po = ps_prj.tile([P, 512], f32, tag="prj", name=f"po_{j}_{m}", bufs=2)
                    nc.tensor.matmul(po[:], wo_r[:, 128 * m : 128 * (m + 1)], attnT[:, sj])
                    ot = spool.tile([P, 512], f32, tag="ot")
                    nc.vector.tensor_copy(ot[:], po[:])
                    nc.sync.dma_start(out_d[128 * m : 128 * (m + 1), sj], ot[:])

    nc.compile()
    return nc


def _prep_inputs(x, wq, bq, wk, bk, wv, bv, wo, bo):
    """Host-side sharding: build the 8 per-core input maps."""
    x = np.ascontiguousarray(x, dtype=np.float32)
    in_maps = []
    for c in range(8):
        b = c // 4
        hb = 2 * (c % 4)
        r0, r1 = 32 * hb, 32 * hb + 64
        xf = x[b].reshape(C, S)
        x2 = np.concatenate([xf[0:128], xf[128:256]], axis=1)  # [128, 8192]

        def wmat(wm):
            # [128, 128]: cols 64*ck + 32*h + d = wm[32*(hb+h)+d, 128*ck + row]
            whT = wm[r0:r1].T  # [256(c), 64(2 heads x 32)]
            return np.concatenate([whT[0:128], whT[128:256]], axis=1)

        bqkf = np.zeros((128, 512), np.float32)
        for h in range(2):
            bqkf[64 * h : 64 * h + 32] = bq[r0 + 32 * h : r0 + 32 * h + 32][:, None]
            bqkf[64 * h + 32 : 64 * h + 64] = bk[r0 + 32 * h : r0 + 32 * h + 32][:, None]
        bvf = np.tile(bv[r0:r1][None, :], (128, 1))

        in_maps.append(
            {
                "x2": np.ascontiguousarray(x2, np.float32),
                "wq_sb": np.ascontiguousarray(wmat(wq), np.float32),
                "wk_sb": np.ascontiguousarray(wmat(wk), np.float32),
                "wv_sb": np.ascontiguousarray(wmat(wv), np.float32),
                "wo_sb": np.ascontiguousarray(wo[:, r0:r1].T, np.float32),
                "bqkf": bqkf,
                "bvf": np.ascontiguousarray(bvf, np.float32),
            }
        )
    return in_maps


def kernel(x, wq, bq, wk, bk, wv, bv, wo, bo, _results_out=None):
    global _BUILT
    from concourse.bass_utils import run_bass_kernel_spmd

    if _BUILT is None:
        _BUILT = _build()
    nc = _BUILT

    x = np.asarray(x, np.float32)
    args = [np.asarray(a, np.float32) for a in (wq, bq, wk, bk, wv, bv, wo, bo)]
    wq, bq, wk, bk, wv, bv, wo, bo = args
    in_maps = _prep_inputs(x, wq, bq, wk, bk, wv, bv, wo, bo)

    res = run_bass_kernel_spmd(nc, in_maps, core_ids=list(range(8)))
    if _results_out is not None:
        _results_out.append(res)

    out = np.zeros((B, C, S), np.float32)
    for c in range(8):
        out[c // 4] += res.results[c]["out"]
    out += bo[None, :, None]
    return out.reshape(B, C, HH, WW)


# revision 12
# speedup vs baseline: 1.1133x; 1.1133x over previous
"""Multi-head attention (B=2, C=256, H=W=64, nh=8) on 8 trn2 NeuronCores.

Sharding: batch*heads = 16 (b,h) pairs -> 2 pairs per core.
Core c handles batch b = c//4 and heads {2*(c%4), 2*(c%4)+1}.
Each core computes q/k/v projections for its heads, transposed-layout
attention (scores kept as [t, s] so softmax needs no transposes), and its
partial output projection; the host sums the 4 per-batch partials + bias.

Layout notes (per core, S = 4096, hd = 32):
  x_b  [128, 8192]  x[b] (bf16) as two 128-channel chunks side by side
  qT4  [128, 4096]  q^T for head h replicated on 4 partition strips
                    (row-packed K=32 scores matmuls need weights and rhs at
                     the same base partition, 32*i for strip i); bf16
  kT4  [128, 4096]  k^T strips: strip i holds t-tiles with tt % 4 == i
                    (kT4[32i:32i+32, 128g:128g+128] = kT[:, 128*(4g+i):...])
  v_sb [128, 2112]  32 t-tiles x 66 cols: [v_h0 | ones | v_h1 | ones] bf16
                    (ones column computes the softmax denominator for free)
  scores psum [128, 2048]: 4 banks, t-tiles 4g..4g+3 of one s-chunk; one
                    Exp ACTIVATE over all 2048 columns -> pT (bf16)
  attn@v: M=33 (v|ones) matmuls accumulated into two psum banks (even/odd
                    t-tiles) so each bank's first write can use start=True
  out  [256, 4096]  partial conv1x1 output for batch b (2 heads' worth),
                    DMAed straight from PSUM
"""

import sys

for _p in ("/opt/trn_rl_repo",):
    if _p not in sys.path:
        sys.path.append(_p)

import numpy as np

B, C, HH, WW = 2, 256, 64, 64
S = HH * WW            # 4096
NH = 8
HD = C // NH           # 32
P = 128
NJ = S // 512          # 8 s-chunks
NT = S // P            # 32 t-tiles
NG = NT // 4           # 8 t-groups
SCALE = 1.0 / np.sqrt(np.float32(HD))

_BUILT = None


def _build():
    import concourse.bass as bass
    import concourse.mybir as mybir
    import concourse.tile as tile
    from concourse import bacc

    dt = mybir.dt
    f32 = dt.float32
    f32r = dt.float32r
    bf16 = dt.bfloat16
    AF = mybir.ActivationFunctionType

    nc = bacc.Bacc("TRN2", target_bir_lowering=False, debug=False, num_devices=8)

    x2 = nc.dram_tensor("x2", [P, 2 * S], f32, kind="ExternalInput")
    wq_d = nc.dram_tensor("wq_sb", [P, 128], f32, kind="ExternalInput")
    wk_d = nc.dram_tensor("wk_sb", [P, 128], f32, kind="ExternalInput")
    wv_d = nc.dram_tensor("wv_sb", [P, 128], f32, kind="ExternalInput")
    wo_d = nc.dram_tensor("wo_sb", [64, 256], f32, kind="ExternalInput")
    bqkf_d = nc.dram_tensor("bqkf", [P, 512], f32, kind="ExternalInput")
    bvf_d = nc.dram_tensor("bvf", [P, 64], f32, kind="ExternalInput")
    out_d = nc.dram_tensor("out", [C, S], f32, kind="ExternalOutput")

    with tile.TileContext(nc) as tc:
        with (
            tc.tile_pool(name="const", bufs=1) as cpool,
            tc.tile_pool(name="qk", bufs=1) as qkpool,
            tc.tile_pool(name="pt", bufs=3) as ptpool,
            tc.tile_pool(name="work", bufs=4) as wpool,
            tc.tile_pool(name="stage", bufs=4) as spool,
            tc.tile_pool(name="ps_sc", bufs=1, space="PSUM") as ps_sc,
            tc.tile_pool(name="ps_acc", bufs=1, space="PSUM") as ps_acc,
            tc.tile_pool(name="ps_prj", bufs=2, space="PSUM") as ps_prj,
        ):
            # ---- constants / inputs in SBUF ----
            x_sb = cpool.tile([P, 2 * S], f32)
            x_b = cpool.tile([P, 2 * S], bf16)
            for ck in range(2):
                for piece in range(2):
                    sl = slice(ck * S + piece * 2048, ck * S + (piece + 1) * 2048)
                    nc.sync.dma_start(x_sb[:, sl], x2[:, sl])
                    nc.vector.tensor_copy(x_b[:, sl], x_sb[:, sl])
            wq_t = cpool.tile([P, 128], f32)
            wk_t = cpool.tile([P, 128], f32)
            wv_t = cpool.tile([P, 128], f32)
            nc.sync.dma_start(wq_t[:], wq_d[:])
            nc.sync.dma_start(wk_t[:], wk_d[:])
            nc.sync.dma_start(wv_t[:], wv_d[:])
            wq_b = cpool.tile([P, 128], bf16)
            wk_b = cpool.tile([P, 128], bf16)
            wv_b = cpool.tile([P, 128], bf16)
            nc.vector.tensor_copy(wq_b[:], wq_t[:])
            nc.vector.tensor_copy(wk_b[:], wk_t[:])
            nc.vector.tensor_copy(wv_b[:], wv_t[:])
            wo_t = cpool.tile([64, 256], f32)
            nc.sync.dma_start(wo_t[:], wo_d[:])
            wo_r = cpool.tile([64, 256], f32r)
            nc.vector.tensor_copy(wo_r[:], wo_t[:])
            bqkf_t = cpool.tile([P, 512], f32)
            bvf_t = cpool.tile([P, 64], f32)
            nc.sync.dma_start(bqkf_t[:], bqkf_d[:])
            nc.sync.dma_start(bvf_t[:], bvf_d[:])

            # ---- v projection: v_sb [128, 32*66] (t on partitions), bf16 ----
            v_sb = cpool.tile([P, NT * 66], bf16)
            ones32 = cpool.tile([P, NT, 1], f32)
            nc.vector.memset(ones32[:], 1.0)
            v3 = v_sb[:].rearrange("p (t c) -> p t c", c=66)
            nc.vector.tensor_copy(v3[:, :, 32:33], ones32[:])
            nc.vector.tensor_copy(v3[:, :, 65:66], ones32[:])
            for tt in range(NT):
                pv = ps_prj.tile([P, 64], f32, tag="prj")
                for ck in range(2):
                    nc.tensor.matmul(
                        pv[:],
                        x_b[:, ck * S + tt * P : ck * S + (tt + 1) * P],
                        wv_b[:, 64 * ck : 64 * ck + 64],
                        start=(ck == 0),
                        stop=(ck == 1),
                    )
                base = tt * 66
                nc.vector.tensor_add(v_sb[:, base : base + 32], pv[:, 0:32], bvf_t[:, 0:32])
                nc.vector.tensor_add(v_sb[:, base + 33 : base + 65], pv[:, 32:64], bvf_t[:, 32:64])

            # ---- q/k projections for both heads: qT4/kT4 [128, 4096] bf16 ----
            # One DVE bias-add into strip 0 (q) / a staging tile (k), then DMA
            # replication/distribution to the other strips (keeps DVE load low).
            qT4 = [qkpool.tile([P, S], bf16, tag=f"qT4_{h}", name=f"qT4_{h}") for h in range(2)]
            kT4 = [qkpool.tile([P, 1024], bf16, tag=f"kT4_{h}", name=f"kT4_{h}") for h in range(2)]
            # per j: one col-packed wave computes q/k for BOTH heads into one
            # psum bank (rows: q_h0 | k_h0 | q_h1 | k_h1), one combined
            # bias-add into a staging buffer; after all j, a few BIG strided
            # DMAs distribute into the qT4/kT4 strips (many small DMAs would
            # drain too slowly and stall the scores matmuls mid-attention)
            qks_all = cpool.tile([P, S], bf16)
            for j in range(NJ):
                pqk = ps_prj.tile([P, 512], f32, tag="prj", name=f"pqk_{j}", bufs=2)
                for ck in range(2):
                    for s4, wsel in enumerate((wq_b, wk_b, wq_b, wk_b)):
                        hcol = 32 * (s4 // 2)
                        nc.tensor.matmul(
                            pqk[32 * s4 : 32 * s4 + 32, :],
                            wsel[:, 64 * ck + hcol : 64 * ck + hcol + 32],
                            x_b[:, ck * S + 512 * j : ck * S + 512 * (j + 1)],
                            tile_position=(0, 32 * s4),
                            start=(ck == 0),
                            stop=(ck == 1),
                        )
                nc.vector.tensor_add(
                    qks_all[:, 512 * j : 512 * (j + 1)], pqk[:], bqkf_t[:]
                )
            qk8 = qks_all[:].rearrange("p (g f) -> p g f", f=512)
            for h in range(2):
                for i in range(4):
                    nc.sync.dma_start(qT4[h][32 * i : 32 * i + 32, :], qks_all[64 * h : 64 * h + 32, :])
                    # kT4 strip i <- k rows, cols 128i..128i+128 of each 512-chunk
                    nc.sync.dma_start(
                        kT4[h][32 * i : 32 * i + 32, :].rearrange("p (g f) -> p g f", f=128),
                        qk8[64 * h + 32 : 64 * h + 64, :, 128 * i : 128 * (i + 1)],
                    )  # kT4 strip cols: one 128-block per t-group g

            # ---- attention + output projection ----
            attnT = cpool.tile([64, S], f32r)
            for j in range(NJ):
                sj = slice(512 * j, 512 * (j + 1))
                for h in range(2):
                    accA = ps_acc.tile([P, 512], f32, tag="accA")
                    accB = ps_acc.tile([P, 512], f32, tag="accB")

                    # half-groups of 2 t-tiles: half (g, u) covers t-tiles
                    # 4g+2u, 4g+2u+1 (strips 2u, 2u+1), psum tags scA/scB
                    # ping-pong so exp of one half overlaps scores of the next
                    def scores_half(g, u):
                        sc = ps_sc.tile(
                            [P, 1024], f32, tag=f"sc{u}", name=f"sc_{h}_{g}_{u}"
                        )
                        for ii in range(2):
                            i = 2 * u + ii
                            nc.tensor.matmul(
                                sc[:, 512 * ii : 512 * (ii + 1)],
                                kT4[h][32 * i : 32 * i + 32, 128 * g : 128 * (g + 1)],
                                qT4[h][32 * i : 32 * i + 32, sj],
                                tile_position=(32 * i, 0),
                            )
                        return sc

                    def attnv_half(g, u, pt):
                        for ii in range(2):
                            tt = 4 * g + 2 * u + ii
                            acc, cpos = (accA, 0) if tt % 2 == 0 else (accB, 64)
                            nc.tensor.matmul(
                                acc[cpos : cpos + 33, :],
                                v_sb[:, tt * 66 + 33 * h : tt * 66 + 33 * h + 33],
                                pt[:, 512 * ii : 512 * (ii + 1)],
                                tile_position=(0, cpos),
                                start=(tt < 2),
                                stop=(tt >= NT - 2),
                            )

                    halves = [(g, u) for g in range(NG) for u in range(2)]
                    sc = scores_half(*halves[0])
                    for idx, (g, u) in enumerate(halves):
                        pt = ptpool.tile([P, 1024], bf16, tag="pt")
                        nc.scalar.activation(pt[:], sc[:], AF.Exp, scale=float(SCALE))
                        if idx + 1 < len(halves):
                            sc = scores_half(*halves[idx + 1])
                        attnv_half(g, u, pt)
                    # combine strips, normalize, write attnT rows for head h
                    numT = wpool.tile([33, 512], f32, tag="numT")
                    nc.vector.tensor_copy(numT[:], accB[64:97, :])
                    nc.vector.tensor_add(numT[:], numT[:], accA[0:33, :])
                    rec = wpool.tile([1, 512], f32, tag="rec")
                    nc.vector.reciprocal(rec[:], numT[32:33, :])
                    bc = wpool.tile([32, 512], f32, tag="bc")
                    nc.gpsimd.partition_broadcast(bc[:], rec[:])
                    nc.vector.tensor_mul(attnT[32 * h : 32 * h + 32, sj], numT[0:32, :], bc[:])

            # ---- output projection tail (attnT fully built) ----
            for j in range(NJ):
                sj = slice(512 * j, 512 * (j + 1))
                for m in range(2):
                    po = ps_prj.tile([P, 512], f32, tag="prj", name=f"po_{j}_{m}", bufs=2)
                    nc.tensor.matmul(po[:], wo_r[:, 128 * m : 128 * (m + 1)], attnT[:, sj])
                    ot = spool.tile([P, 512], f32, tag="ot")
                    nc.vector.tensor_copy(ot[:], po[:])
                    nc.sync.dma_start(out_d[128 * m : 128 * (m + 1), sj], ot[:])

    nc.compile()
    return nc


def _prep_inputs(x, wq, bq, wk, bk, wv, bv, wo, bo):
    """Host-side sharding: build the 8 per-core input maps."""
    x = np.ascontiguousarray(x, dtype=np.float32)
    in_maps = []
    for c in range(8):
        b = c // 4
        hb = 2 * (c % 4)
        r0, r1 = 32 * hb, 32 * hb + 64
        xf = x[b].reshape(C, S)
        x2 = np.concatenate([xf[0:128], xf[128:256]], axis=1)  # [128, 8192]

        def wmat(wm):
            # [128, 128]: cols 64*ck + 32*h + d = wm[32*(hb+h)+d, 128*ck + row]
            whT = wm[r0:r1].T  # [256(c), 64(2 heads x 32)]
            return np.concatenate([whT[0:128], whT[128:256]], axis=1)

        bqkf = np.zeros((128, 512), np.float32)
        for h in range(2):
            bqkf[64 * h : 64 * h + 32] = bq[r0 + 32 * h : r0 + 32 * h + 32][:, None]
            bqkf[64 * h + 32 : 64 * h + 64] = bk[r0 + 32 * h : r0 + 32 * h + 32][:, None]
        bvf = np.tile(bv[r0:r1][None, :], (128, 1))

        in_maps.append(
            {
                "x2": np.ascontiguousarray(x2, np.float32),
                "wq_sb": np.ascontiguousarray(wmat(wq), np.float32),
                "wk_sb": np.ascontiguousarray(wmat(wk), np.float32),
                "wv_sb": np.ascontiguousarray(wmat(wv), np.float32),
                "wo_sb": np.ascontiguousarray(wo[:, r0:r1].T, np.float32),
                "bqkf": bqkf,
                "bvf": np.ascontiguousarray(bvf, np.float32),
            }
        )
    return in_maps


def kernel(x, wq, bq, wk, bk, wv, bv, wo, bo, _results_out=None):
    global _BUILT
    from concourse.bass_utils import run_bass_kernel_spmd

    if _BUILT is None:
        _BUILT = _build()
    nc = _BUILT

    x = np.asarray(x, np.float32)
    args = [np.asarray(a, np.float32) for a in (wq, bq, wk, bk, wv, bv, wo, bo)]
    wq, bq, wk, bk, wv, bv, wo, bo = args
    in_maps = _prep_inputs(x, wq, bq, wk, bk, wv, bv, wo, bo)

    res = run_bass_kernel_spmd(nc, in_maps, core_ids=list(range(8)))
    if _results_out is not None:
        _results_out.append(res)

    out = np.zeros((B, C, S), np.float32)
    for c in range(8):
        out[c // 4] += res.results[c]["out"]
    out += bo[None, :, None]
    return out.reshape(B, C, HH, WW)
